# revision 28
# baseline (speedup 1.0000x reference)
"""Causal self-attention (B=2, T=2048, C=1024, H=16) on 8 trn2 NeuronCores.

Sharding: data-parallel over batch (2 groups of 4 cores) x tensor-parallel over
heads (4 heads / core). v2 layout: single overlapped schedule (no phase
barriers), PV matmul in [query, channel] orientation with bf16 P/V (65-row
matmuls), per-partition softmax division on Pool, DVE/PE transposes back to
yT, bf16 output projection, and the group ReduceScatter split into 4
per-token-chunk bf16 collectives issued as soon as each projection chunk is
done so communication overlaps compute.
"""

import sys

for _p in ("/opt/trn_rl_repo",):
    if _p not in sys.path:
        sys.path.append(_p)

import numpy as np
from contextlib import ExitStack

import concourse.bass as bass
import concourse.mybir as mybir
import concourse.tile as tile
from concourse import bass_utils

B, T, C, H = 2, 2048, 1024, 16
D = C // H              # 64
N_CORES = 8
GROUPS = [[0, 1, 2, 3], [4, 5, 6, 7]]
HL = 4                  # heads per core
CL = HL * D             # 256 local channels
KC = C // 128           # 8 contraction chunks of 128
NT = T // 512           # 4 token chunks of 512
TOKC = T // 128         # 16 token chunks of 128
F32 = mybir.dt.float32
F32R = mybir.dt.float32r
BF16 = mybir.dt.bfloat16


def _legalize_waits(nc):
    """This walrus build allows at most ONE sync-wait per instruction. Move
    extra waits onto same-engine NoOps inserted just before the instruction."""
    n_split = 0
    for f in nc.m.functions:
        for bb in f.blocks:
            out = []
            for inst in bb.instructions:
                si = inst.sync_info
                waits = list(si.on_wait) if si is not None and si.on_wait else []
                if len(waits) > 1:
                    for i, w in enumerate(waits[:-1]):
                        out.append(
                            mybir.InstNoOp(
                                name=f"wsplit_{inst.name}_{i}",
                                engine=inst.engine,
                                ins=[],
                                outs=[],
                                sync_info=mybir.SyncInfo(on_wait=[w], on_update=[]),
                            )
                        )
                        n_split += 1
                    si.on_wait = [waits[-1]]
                out.append(inst)
            bb.instructions = out
    return n_split


def _build_bass():
    nc = bass.Bass("TRN2", target_bir_lowering=False, debug=False,
                   num_devices=N_CORES)

    xT = nc.dram_tensor("xT", [C, T], BF16, kind="ExternalInput").ap()
    w_qk = nc.dram_tensor("w_qk", [C, 2 * CL], BF16, kind="ExternalInput").ap()
    b_qk = nc.dram_tensor("b_qk", [2 * CL], F32, kind="ExternalInput").ap()
    w_v = nc.dram_tensor("w_v", [C, CL], BF16, kind="ExternalInput").ap()
    b_v = nc.dram_tensor("b_v", [CL], F32R, kind="ExternalInput").ap()
    w_pr = nc.dram_tensor("w_pr", [CL, C], BF16, kind="ExternalInput").ap()
    b_pr = nc.dram_tensor("b_pr", [C], F32, kind="ExternalInput").ap()
    out_rs = nc.dram_tensor("out_rs", [C // 4, T], BF16, kind="ExternalOutput").ap()

    with tile.TileContext(nc) as tc:
        with ExitStack() as ctx:
            with nc.allow_low_precision(reason="bf16 attention/proj; tolerance 2e-2"):
                _build_body(ctx, tc, nc, xT, w_qk, b_qk, w_v, b_v, w_pr, b_pr, out_rs)

    _legalize_waits(nc)
    return nc


def _build_body(ctx, tc, nc, xT, w_qk, b_qk, w_v, b_v, w_pr, b_pr, out_rs):
    Exp = mybir.ActivationFunctionType.Exp

    persist = ctx.enter_context(tc.tile_pool(name="persist", bufs=1))
    dram = ctx.enter_context(tc.tile_pool(name="dram", bufs=1, space="DRAM"))

    # ---- weight + activation loads, ordered so phase A starts earliest ----
    b_qk_sb = persist.tile([128, 4], F32, name="b_qk_sb")
    nc.sync.dma_start(b_qk_sb[:], b_qk.rearrange("(m p) -> p m", p=128))
    # 4 contraction chunks per DMA: fewer HWDGE slots gate the start
    w_qk_m = w_qk.rearrange("(g k p) c -> g p k c", g=2, k=4)
    w_qk_t = []
    for g in range(2):
        t = persist.tile([128, 4 * 2 * CL], BF16, name=f"w_qk_m{g}")
        eng = nc.sync if g == 0 else nc.scalar
        eng.dma_start(t[:].rearrange("p (k c) -> p k c", k=4), w_qk_m[g])
        w_qk_t.append(t)

    class _Slices:
        def __init__(self, tiles, width):
            self.tiles, self.width = tiles, width

        def __getitem__(self, kc):
            t = self.tiles[kc // 4]
            o = (kc % 4) * self.width
            return t[:, o:o + self.width]

    w_qk_sb = _Slices(w_qk_t, 2 * CL)
    xT_m = xT.rearrange("(g k p) (n c) -> n g p k c", g=2, k=4, n=NT)
    xT_t = {}
    xT_sb = {}

    def load_x(n):
        for g in range(2):
            t = persist.tile([128, 4 * 512], BF16, name=f"xT_m{g}_{n}")
            eng = nc.scalar if (g == 1 and n == 0) else nc.sync
            eng.dma_start(t[:].rearrange("p (k c) -> p k c", k=4), xT_m[n, g])
            xT_t[g, n] = t
        for kc in range(KC):
            g, o = kc // 4, (kc % 4) * 512
            xT_sb[kc, n] = xT_t[g, n][:, o:o + 512]

    load_x(0)
    b_v_row = persist.tile([1, CL], F32R, name="b_v_row")
    nc.sync.dma_start(b_v_row[:], b_v.rearrange("(a c) -> a c", a=1))
    w_v_m = w_v.rearrange("(g k p) c -> g p k c", g=2, k=4)
    w_v_t = []
    for g in range(2):
        t = persist.tile([128, 4 * CL], BF16, name=f"w_v_m{g}")
        nc.sync.dma_start(t[:].rearrange("p (k c) -> p k c", k=4), w_v_m[g])
        w_v_t.append(t)
    w_v_sb = _Slices(w_v_t, CL)
    load_x(1)
    w_prb = []
    for kc in range(2):
        t = persist.tile([128, C], BF16, name=f"w_prb_{kc}")
        nc.sync.dma_start(t[:], w_pr[kc * 128:(kc + 1) * 128, :])
        w_prb.append(t)
    load_x(2)
    load_x(3)
    b_pr_sb = persist.tile([128, 8], F32, name="b_pr_sb")
    nc.sync.dma_start(b_pr_sb[:], b_pr.rearrange("(m p) -> p m", p=128))
    ones_f32 = persist.tile([1, 128], F32, name="ones_f32")
    nc.gpsimd.memset(ones_f32[:], 1.0)
    ones_row = persist.tile([1, 128], F32R, name="ones_row")
    nc.vector.tensor_copy(ones_row[:], ones_f32[:])

    # 0/1 bf16 triangle mask for the diagonal 128x128 block: tri[k,q]=(q>=k)
    tri = persist.tile([128, 128], BF16, name="tri")
    nc.gpsimd.memset(tri[:], 1.0)
    nc.gpsimd.affine_select(
        out=tri[:], in_=tri[:], compare_op=mybir.AluOpType.is_ge, fill=0.0,
        base=0, pattern=[[1, 128]], channel_multiplier=-1)

    # bf16 identity for PE transposes
    ident = persist.tile([128, 128], BF16, name="ident")
    nc.gpsimd.memset(ident[:], 1.0)
    nc.gpsimd.affine_select(
        out=ident[:], in_=ident[:], compare_op=mybir.AluOpType.is_ge, fill=0.0,
        base=0, pattern=[[1, 128]], channel_multiplier=-1)
    nc.gpsimd.affine_select(
        out=ident[:], in_=ident[:], compare_op=mybir.AluOpType.is_ge, fill=0.0,
        base=0, pattern=[[-1, 128]], channel_multiplier=1)

    # ---- persistent intermediates ----------------------------------------
    # QK_sb[m]: m=0,1 -> Q channels (heads 0,1 | 2,3), m=2,3 -> K channels
    QK_sb = [persist.tile([128, T], F32R, name=f"QK_{m}") for m in range(4)]
    # V in [tok, ch] bf16 layout, 65 cols/head: col h*65+64 is the ones column
    V_sb = [persist.tile([128, HL * 65], BF16, name=f"V_{t}") for t in range(TOKC)]
    # y^T in bf16, rows = local channels (kc 0: heads 0,1; kc 1: heads 2,3)
    yT_sb = [persist.tile([128, T], BF16, name=f"yT_{i}") for i in range(2)]

    # ones columns of V (written once; disjoint from the per-head 64-col data)
    for t in range(TOKC):
        vt = V_sb[t][:].rearrange("p (h c) -> p h c", h=HL)
        nc.gpsimd.memset(vt[:, :, 64:65], 1.0)

    # broadcast b_v across partitions via a rank-1 matmul
    b_v_bc = persist.tile([128, CL], F32, name="b_v_bc")

    # ---- pools (kept open for the whole kernel; no phase barriers) --------
    psW = ctx.enter_context(tc.tile_pool(name="psW", bufs=2, space="PSUM"))
    psB = ctx.enter_context(tc.tile_pool(name="psB", bufs=1, space="PSUM"))
    psS = ctx.enter_context(tc.tile_pool(name="psS", bufs=2, space="PSUM"))
    psPV = ctx.enter_context(tc.tile_pool(name="psPV", bufs=2, space="PSUM"))
    psT = ctx.enter_context(tc.tile_pool(name="psT", bufs=1, space="PSUM"))
    pP = ctx.enter_context(tc.tile_pool(name="pP", bufs=34))
    pR = ctx.enter_context(tc.tile_pool(name="pR", bufs=2))
    pY = ctx.enter_context(tc.tile_pool(name="pY", bufs=10))
    pO = ctx.enter_context(tc.tile_pool(name="pO", bufs=4))

    bvp = psW.tile([128, 512], F32, name="a_ps")
    nc.tensor.matmul(bvp[:, 0:CL], lhsT=ones_row[:], rhs=b_v_row[:],
                     start=True, stop=True)
    nc.vector.tensor_copy(b_v_bc[:], bvp[:, 0:CL])

    bounce_n = [dram.tile([C, 512], BF16, name=f"bounce_{n}") for n in range(NT)]
    rs_out_n = [dram.tile([C // 4, 512], BF16, name=f"rs_out_{n}") for n in range(NT)]

    def q_ap(h):
        return QK_sb[h // 2][(h % 2) * 64:(h % 2) * 64 + 64, :]

    def k_ap(h):
        return QK_sb[2 + h // 2][(h % 2) * 64:(h % 2) * 64 + 64, :]

    # ---- static scheduler: PE is the master stream; exps (Act) must never
    # starve.  Fillers are single matmuls injected whenever the PE virtual
    # clock is ahead of the Act virtual clock.
    PE_ROW = 1.0 / 2.4          # ns per output row at full p-state
    ACT_ROW = 1.0 / 1.2
    ACT_FIX = 185.0
    EXP_LAT = 250.0             # sem hop from score-done to exp start

    state = {"pe": 0.0, "act": 0.0}
    fillers = []                # list of (cost_ns, closure)

    def emit_fillers(margin=800.0):
        if state["act"] == 0.0:
            return
        while fillers and state["pe"] < state["act"] + margin:
            cost, run, _tag = fillers.pop(0)
            run()
            state["pe"] += cost

    def drain_fillers(upto_tag=None):
        while fillers and (upto_tag is None or fillers[0][2] is not None
                           and fillers[0][2] <= upto_tag):
            cost, run = fillers.pop(0)[:2]
            run()
            state["pe"] += cost

    # A(m,n): 8 kc-matmuls accumulating into one psW tile, then DVE bias-add.
    # m order 0,2 (heads 0/1 Q+K) then 1,3 so attention can start earliest.
    def add_a_fillers(n):
        holder = {}
        for gi, grp in enumerate(((0, 2), (1, 3))):
            for m in grp:
                def mk(m, kc):
                    def run():
                        if kc == 0:
                            holder[m] = psW.tile([128, 512], F32, name="a_ps")
                        nc.tensor.matmul(
                            holder[m][:], lhsT=w_qk_sb[kc][:, m * 128:(m + 1) * 128],
                            rhs=xT_sb[kc, n], start=(kc == 0), stop=(kc == KC - 1))
                        if kc == KC - 1:
                            nc.vector.tensor_scalar_add(
                                QK_sb[m][:, n * 512:(n + 1) * 512], holder.pop(m)[:],
                                b_qk_sb[:, m:m + 1])
                    return run
                for kc in range(KC):
                    fillers.append((213.0, mk(m, kc), 3 * n + gi))

    # B(t,n): 8 kc-matmuls into psB, then Pool bias-add into V bf16.
    def add_b_fillers(n):
        holder = {}
        for t in range(4):
            def mk(t, kc):
                def run():
                    if kc == 0:
                        holder[t] = psB.tile([128, CL], F32, name="b_ps")
                    nc.tensor.matmul(
                        holder[t][:], lhsT=xT_sb[kc, n][:, t * 128:(t + 1) * 128],
                        rhs=w_v_sb[kc][:], start=(kc == 0), stop=(kc == KC - 1))
                    if kc == KC - 1:
                        vt = V_sb[n * 4 + t][:].rearrange("p (h c) -> p h c", h=HL)
                        nc.vector.tensor_add(
                            vt[:, :, 0:64],
                            holder.pop(t)[:].rearrange("p (h c) -> p h c", h=HL),
                            b_v_bc[:].rearrange("p (h c) -> p h c", h=HL))
                return run
            for kc in range(KC):
                fillers.append((107.0, mk(t, kc), 3 * n + 2))

    # D(m,qi): 2 kc-matmuls + DVE bias-add + DMA; last m also issues the RS.
    def d_closures(qi, alt=False):
        out = []
        for m in range(8):
            def mk(m):
                def run():
                    if alt and m % 2 == 1:
                        ps = psS.tile([128, 512], F32, name="s_ps")
                    else:
                        ps = psW.tile([128, 512], F32, name="a_ps")
                    for kc in range(2):
                        nc.tensor.matmul(
                            ps[:], lhsT=w_prb[kc][:, m * 128:(m + 1) * 128],
                            rhs=yT_sb[kc][:, qi * 512:(qi + 1) * 512],
                            start=(kc == 0), stop=(kc == 1))
                    o_sb = pO.tile([128, 512], BF16)
                    nc.vector.tensor_scalar_add(o_sb[:], ps[:],
                                                b_pr_sb[:, m:m + 1])
                    eng = nc.scalar if (alt and m % 2 == 1) else nc.sync
                    eng.dma_start(
                        bounce_n[qi][m * 128:(m + 1) * 128, :], o_sb[:])
                    if m == 7:
                        nc.gpsimd.collective_compute(
                            "ReduceScatter", mybir.AluOpType.add,
                            replica_groups=GROUPS,
                            ins=[bounce_n[qi][:]], outs=[rs_out_n[qi][:]])
                        nc.sync.dma_start(
                            out_rs[:, qi * 512:(qi + 1) * 512], rs_out_n[qi][:])
                return run
            out.append(mk(m))
        return out

    # ---- main pipeline ----------------------------------------------------
    add_a_fillers(0)
    add_b_fillers(0)
    add_a_fillers(1)
    add_b_fillers(1)
    add_a_fillers(2)
    add_b_fillers(2)
    add_a_fillers(3)
    add_b_fillers(3)

    carry_d = []
    for qi in (0, 1, 2, 3):
        drain_fillers(upto_tag=3 * qi)       # A of heads 0/1 for chunks <= qi
        nch = 4 * (qi + 1)
        gi = 0
        tail_q = []
        for h in range(HL):
            if h == 2:
                drain_fillers(upto_tag=3 * qi + 1)
            p_tiles = []
            y_ps = psPV.tile([128, 4 * 65], F32)
            for j in range(nch):
                rel = j - 4 * qi
                off = 128 * rel if rel >= 0 else 0
                s_ps = psS.tile([128, 512], F32)
                nc.tensor.matmul(
                    s_ps[:, off:], lhsT=k_ap(h)[:, j * 128:(j + 1) * 128],
                    rhs=q_ap(h)[:, qi * 512 + off:(qi + 1) * 512],
                    start=True, stop=True)
                rows = 512 - off
                state["pe"] += rows * PE_ROW
                state["act"] = max(state["act"], state["pe"] + EXP_LAT) \
                    + rows * ACT_ROW + ACT_FIX
                p_sb = pP.tile([128, 512], BF16)
                nc.scalar.activation(p_sb[:, off:], s_ps[:, off:], Exp,
                                     scale=0.125)
                if rel >= 0:
                    nc.vector.tensor_mul(
                        p_sb[:, off:off + 128], p_sb[:, off:off + 128], tri[:])
                if h == 0 and j == 4 * qi:
                    drain_fillers(upto_tag=3 * qi + 2)   # V of chunk qi
                p_tiles.append(p_sb)
                if tail_q:
                    tail_q.pop(0)()
                emit_fillers()
                gi += 1
                if carry_d and gi % 2 == 0:
                    carry_d.pop(0)()
                    state["pe"] += 426.0
                if h == 3 and fillers and fillers[0][2] <= 3 * qi + 5:
                    cost, run, _t = fillers.pop(0)
                    run()
                    state["pe"] += cost
            # defer this head's PV burst + epilogue; each closure is one
            # complete PSUM accumulation group (kept consecutive) or the
            # divide/transpose chain, popped during the next head's j-loop
            def mk_pv(h, y_ps, p_tiles, s):
                def run():
                    for j in range(4 * qi + s + 1):
                        nc.tensor.matmul(
                            y_ps[:, 65 * s:65 * s + 65],
                            lhsT=p_tiles[j][:, 128 * s:128 * s + 128],
                            rhs=V_sb[j][:, 65 * h:65 * h + 65],
                            start=(j == 0), stop=(j == 4 * qi + s))
                        state["pe"] += 27.0
                return run

            def mk_ep(h, y_ps):
                def run():
                    rec = pR.tile([128, 4], F32)
                    nc.vector.reciprocal(
                        rec[:],
                        y_ps[:].rearrange("p (s c) -> p s c", s=4)[:, :, 64])
                    kcb = h // 2
                    row0 = (h % 2) * 64
                    for s in range(4):
                        y_sb = pY.tile([128, 64], BF16)
                        nc.vector.tensor_scalar_mul(
                            y_sb[:], y_ps[:, 65 * s:65 * s + 64],
                            rec[:, s:s + 1])
                        t_ps = psT.tile([64, 128], BF16)
                        nc.tensor.transpose(t_ps[:], y_sb[:], ident[:])
                        state["pe"] += 53.0
                        nc.vector.tensor_copy(
                            yT_sb[kcb][row0:row0 + 64,
                                       qi * 512 + 128 * s:
                                       qi * 512 + 128 * s + 128],
                            t_ps[:])
                return run

            for s in range(4):
                tail_q.append(mk_pv(h, y_ps, p_tiles, s))
            tail_q.append(mk_ep(h, y_ps))
        for run in tail_q:
            run()
        for run in carry_d:
            run()
        carry_d = d_closures(qi, alt=(qi == NT - 1))
    for run in carry_d:
        run()
    drain_fillers()


_NC_CACHE = None


def _get_nc():
    global _NC_CACHE
    if _NC_CACHE is None:
        _NC_CACHE = _build_bass()
    return _NC_CACHE


def kernel(x, w_qkv, b_qkv, w_proj, b_proj, **_kw):
    x = np.asarray(x, dtype=np.float32)
    w_qkv = np.asarray(w_qkv, dtype=np.float32)
    b_qkv = np.asarray(b_qkv, dtype=np.float32)
    w_proj = np.asarray(w_proj, dtype=np.float32)
    b_proj = np.asarray(b_proj, dtype=np.float32)

    nc = _get_nc()
    in_maps = []
    for c in range(N_CORES):
        b = c // 4
        g = c % 4
        qs = slice(g * CL, (g + 1) * CL)
        ks = slice(C + g * CL, C + (g + 1) * CL)
        vs = slice(2 * C + g * CL, 2 * C + (g + 1) * CL)
        import ml_dtypes
        bf = ml_dtypes.bfloat16
        in_maps.append({
            "xT": np.ascontiguousarray(x[b].T.astype(bf)),
            "w_qk": np.ascontiguousarray(
                np.concatenate([w_qkv[:, qs], w_qkv[:, ks]], axis=1).astype(bf)),
            "b_qk": np.ascontiguousarray(
                np.concatenate([b_qkv[qs], b_qkv[ks]])),
            "w_v": np.ascontiguousarray(w_qkv[:, vs].astype(bf)),
            "b_v": np.ascontiguousarray(b_qkv[vs]),
            "w_pr": np.ascontiguousarray(w_proj[g * CL:(g + 1) * CL, :].astype(bf)),
            "b_pr": b_proj if g == 0 else np.zeros_like(b_proj),
        })

    res = bass_utils.run_bass_kernel_spmd(nc, in_maps, core_ids=list(range(N_CORES)))

    out = np.empty((B, T, C), dtype=np.float32)
    for b in range(B):
        projT = np.concatenate(
            [np.asarray(res.results[4 * b + r]["out_rs"], dtype=np.float32)
             for r in range(4)], axis=0)
        out[b] = projT.T
    return out


if __name__ == "__main__":
    rng = np.random.RandomState(0)
    ins = {
        "x": rng.randn(B, T, C).astype(np.float32),
        "w_qkv": rng.randn(C, 3 * C).astype(np.float32) / 32,
        "b_qkv": rng.randn(3 * C).astype(np.float32) / 32,
        "w_proj": rng.randn(C, C).astype(np.float32) / 32,
        "b_proj": rng.randn(C).astype(np.float32) / 32,
    }
    y = kernel(**ins)
    print("kernel ran, out shape", y.shape)
